# revision 33
# baseline (speedup 1.0000x reference)
"""Multi-head causal attention (B=2, T=2048, D=1024, H=16) on 8 TRN2 cores.

Sharding: core = 4*b + g handles batch b, heads 4g..4g+3 (head/tensor
parallel).  Each core computes its 4 heads end-to-end plus a partial
out-projection; the host sums the 4 partials per batch and adds biases
that commute with the linear ops (b_out, and b_v folded through W_out).

Per-core dataflow (all PE matmuls bf16, f32 PSUM accumulate):
  xT    = x_b^T                             transposed on HOST, plain DMA
  qkT   = (Wqk xT) + b_qk                   feature-major [e, t]
  v     = xT^T Wv^T                         token-major  [t, e_v] with ones cols
  sT_i  = kT_h[:,tk_i]^T qT_h               scoresT [tk, tq], row-packed head
                                            pairs; hh1 written base-512 so the
                                            exp input is one contiguous span
  mask  = step^T (-30000 I)                 causal mask accumulated in PSUM on
                                            diagonal blocks (exp underflows->0)
  e_i   = exp(0.125 * sT_i)                 ACT, one op per group
  paU   = sum_i [v_i | 1]^T e_i             PV matmul; rows 64:128 = Z replicated
  attnT = paU[0:64] * recip(paU[64:128])    deferred softmax normalization
  y     = attnT^T WoT                       partial out-projection [t, e_out]

PE stream is organized as per-group bursts [scores(g) | PV(g-2)] that share
one semaphore wait so matmuls pipeline back-to-back and HAM stays warm.
"""

import os
import sys

sys.path.insert(0, "/opt/trn_rl_repo")

import numpy as np
import ml_dtypes

import concourse.bass as bass
import concourse.mybir as mybir
from concourse.bass_utils import run_bass_kernel_spmd

dt = mybir.dt
F32, BF16 = dt.float32, dt.bfloat16
bf = ml_dtypes.bfloat16
AF = mybir.ActivationFunctionType
ALU = mybir.AluOpType

B, T, D, H = 2, 2048, 1024, 16
HD = D // H                 # 64
HPC = 4                     # heads per core
N_CORES = 8
TB = T // 128               # 16 token blocks
DB = D // 128               # 8 feature blocks of x
NC_CHUNK = 512              # tq chunk
NJ = T // NC_CHUNK          # 4 chunks
SCALE = 1.0 / np.sqrt(HD)   # 0.125
NEG = -30000.0


class Emit:
    """Tracks semaphore counts on the python side while emitting."""

    def __init__(self, nc):
        self.nc = nc
        self.last_wait = {}

    def wge(self, eng, sem, val):
        """Standalone 1-wait instruction; skipped if this engine already
        waited for >= val on this sem."""
        if val <= 0:
            return
        key = (id(eng), id(sem))
        if self.last_wait.get(key, -1) >= val:
            return
        self.last_wait[key] = val
        eng.wait_ge(sem, val)


def build_nc():
    nc = bass.Bass()

    xt_d = nc.dram_tensor("xt", [D, T], BF16, kind="ExternalInput")
    wqk_d = nc.dram_tensor("wqk", [D, 512], BF16, kind="ExternalInput")
    wv_d = nc.dram_tensor("wv", [D, 256], BF16, kind="ExternalInput")
    wo_d = nc.dram_tensor("wo", [256, D], BF16, kind="ExternalInput")
    bqk_d = nc.dram_tensor("bqk", [128, 4], F32, kind="ExternalInput")
    msk_d = nc.dram_tensor("msk", [128, 256], BF16, kind="ExternalInput")
    y_d = nc.dram_tensor("y", [T, D], BF16, kind="ExternalOutput")

    from contextlib import ExitStack

    ctx = ExitStack()
    sem = lambda n: ctx.enter_context(nc.semaphore(n))
    sb = lambda n, s, t: ctx.enter_context(nc.sbuf_tensor(n, s, t))
    psum = lambda n, s: ctx.enter_context(nc.psum_tensor(n, s, F32))

    sLWe = [sem(f"sLW{eb}") for eb in range(4)]   # one per wqk eb load
    sWV = sem("sWV")    # wv load +16
    sBQ = sem("sBQ")    # bqk load +16
    sMQ = sem("sMQ")    # msk load +16
    sWO = sem("sWO")    # wo load +16
    sXT = [sem(f"sXT{j}") for j in range(NJ)]     # one per xT chunk load
    sPR = sem("sPR")    # Q-proj psum groups done (PE), +1
    sYP = sem("sYP")    # O-proj psum groups done (PE), +1
    sPP = sem("sPP")    # unified pp-psum consumer count (DVE), +1
    sSC = sem("sSC")    # score groups done incl. mask MMs (PE), +1 per group
    sEX = sem("sEX")    # exp groups done (ACT), +1
    sPV = sem("sPV")    # PV matmuls done, +1
    sRC = sem("sRC")    # ACT recip done, +1 per (pair,J,hh)
    sNM = sem("sNM")    # normalized attnT written (DVE), +1 per (pair,J,hh)
    sST = sem("sST")    # y stores (POOL), +16

    xT = sb("xT", [128, DB * T], BF16)                # 32KB/part
    wqk_s = sb("wqk_s", [128, DB * 512], BF16)        # 8KB/part
    wv_s = sb("wv_s", [128, DB * 256], BF16)          # 4KB/part
    wo_s = sb("wo_s", [128, 2 * D], BF16)             # 4KB/part
    bqk_s = sb("bqk_s", [128, 4], F32)
    msk_s = sb("msk_s", [128, 256], BF16)             # step | -30000*I
    qk_s = sb("qk_s", [128, 4 * T], BF16)             # 16KB/part
    vo_s = sb("vo_s", [128, TB * 512], BF16)          # 16KB/part
    ex_s = sb("ex_s", [128, 6 * 1024], BF16)          # 6 ring slots of [128,1024]
    rz_s = sb("rz_s", [128, 512], F32)                # hh0 rows 0:64, hh1 64:128
    scr_s = sb("scr_s", [1, 2], F32)                  # dummy-recip scratch
    at_s = sb("at_s", [128, 2 * T], BF16)             # 8KB/part
    y_s = sb("y_s", [128, 2 * 512], BF16)             # 2 slots

    pp = [psum("pp0", [128, 512]), psum("pp1", [128, 512])]
    ps2 = [psum("ps2a", [128, 1024]), psum("ps2b", [128, 1024])]
    pa = [psum("pa0", [128, 512]), psum("pa1", [128, 512])]

    em = Emit(nc)
    PE, ACT, DVE, SP, POOL = nc.tensor, nc.scalar, nc.vector, nc.sync, nc.gpsimd

    def act_recip(out, in_):
        inputs = [ACT.lower_ap(in_)]
        for arg in (0.0, 1.0, 0.0):  # bias, scale, alpha
            inputs.append(mybir.ImmediateValue(dtype=mybir.dt.float32, value=arg))
        return ACT.add_instruction(mybir.InstActivation(
            name=nc.get_next_instruction_name(),
            func=AF.Reciprocal, ins=inputs, outs=[ACT.lower_ap(out)]))

    # ------------------------------------------------------------- DMA loads
    # everything on SP (HWDGE -- much cheaper descriptors than POOL's SWDGE),
    # ordered by first need: x chunk 0, wqk blocks, wv, bqk, msk, x rest, wo.
    xT_3d = xT[:, :].rearrange("p (db t) -> p db t", db=DB)
    xt_src = xt_d[:, :].rearrange("(db p) t -> p db t", p=128)
    SP.dma_start(xT_3d[:, :, 0:512], xt_src[:, :, 0:512]).then_inc(sXT[0], 16)

    # wqk split per eb block so the first qk-proj group can start after just
    # 256KB of weights (+ x chunk 0) instead of the full 1MB.
    wqk_src = wqk_d[:, :].rearrange("(db p) e -> p db e", p=128)
    wqk_dst = wqk_s[:, :].rearrange("p (db e) -> p db e", db=DB)
    for eb in range(4):
        SP.dma_start(
            wqk_dst[:, :, eb * 128:(eb + 1) * 128],
            wqk_src[:, :, eb * 128:(eb + 1) * 128],
        ).then_inc(sLWe[eb], 16)
    SP.dma_start(
        wv_s[:, :].rearrange("p (db e) -> p db e", db=DB),
        wv_d[:, :].rearrange("(db p) e -> p db e", p=128),
    ).then_inc(sWV, 16)
    SP.dma_start(bqk_s[:, :], bqk_d[:, :]).then_inc(sBQ, 16)
    SP.dma_start(msk_s[:, :], msk_d[:, :]).then_inc(sMQ, 16)
    for J in range(1, NJ):
        SP.dma_start(
            xT_3d[:, :, J * 512:(J + 1) * 512],
            xt_src[:, :, J * 512:(J + 1) * 512],
        ).then_inc(sXT[J], 16)
    SP.dma_start(
        wo_s[:, :].rearrange("p (fb e) -> p fb e", fb=2),
        wo_d[:, :].rearrange("(fb p) e -> p fb e", p=128),
    ).then_inc(sWO, 16)

    # ------------------------------------------------- emission helper state
    n = dict(pr=0, yp=0, ppu=0, sc=0, ex=0, pv=0, rc=0, nm=0, st=0)
    gidx = [0]              # global score/exp group index
    ex_after_g = {}         # gidx -> sEX after that group
    pv_after_g = {}         # gidx -> sPV after PV(g) emitted
    qk_done = {}            # chunk J -> sPP after its 4 qk-proj copies
    v_done = {}             # tb -> sPP after its v copy
    nm_after = {}           # (pair, J) -> sNM count
    nm_prev = [0, 0]        # sNM count freeing pa[hh]

    fill_q = []             # pending filler closures (each = PE half-group)

    pend = {}               # state shared between the two halves of a group

    def emit_q_half(J, kind, idx, half):
        """Half of a projection psum group: 4 PE MMs; 2nd half adds DVE copy."""
        if half == 0:
            pend[(kind, idx)] = n["ppu"]
            n["ppu"] += 1
            u = pend[(kind, idx)]
            if kind == "qk":
                em.wge(PE, sLWe[idx], 16)
            else:
                em.wge(PE, sWV, 16)
            em.wge(PE, sXT[J], 16)
            em.wge(PE, sPP, u - 1)
        u = pend[(kind, idx)]
        dbs = range(4) if half == 0 else range(4, 8)
        if kind == "qk":
            eb = idx
            for db in dbs:
                mm = PE.matmul(
                    pp[u % 2][:, :],
                    wqk_s[:, db * 512 + eb * 128: db * 512 + (eb + 1) * 128],
                    xT[:, db * T + J * 512: db * T + (J + 1) * 512],
                    start=(db == 0), stop=(db == DB - 1))
        else:
            tb = idx
            for db in dbs:
                mm = PE.matmul(
                    pp[u % 2][:, 0:256],
                    xT[:, db * T + tb * 128: db * T + (tb + 1) * 128],
                    wv_s[:, db * 256:(db + 1) * 256],
                    start=(db == 0), stop=(db == DB - 1))
        if half == 0:
            return
        mm.then_inc(sPR, 1)
        n["pr"] += 1
        # DVE consumer
        em.wge(DVE, sPR, n["pr"])
        if kind == "qk":
            eb = idx
            em.wge(DVE, sBQ, 16)
            DVE.tensor_scalar(
                qk_s[:, eb * T + J * 512: eb * T + (J + 1) * 512],
                pp[u % 2][:, :], bqk_s[:, eb:eb + 1], None,
                op0=ALU.add).then_inc(sPP, 1)
        else:
            tb = idx
            DVE.tensor_copy(
                vo_s[:, tb * 512:(tb + 1) * 512]
                .rearrange("p (h e) -> p h e", h=4)[:, :, 0:64],
                pp[u % 2][:, 0:256].rearrange("p (h e) -> p h e", h=4),
            ).then_inc(sPP, 1)
        if kind == "qk":
            qk_left[J] -= 1
            if qk_left[J] == 0:
                qk_done[J] = n["ppu_copies"] + 1
        else:
            v_done[idx] = n["ppu_copies"] + 1
        n["ppu_copies"] += 1

    n["ppu_copies"] = 0
    qk_left = {}

    def emit_o_group(J, tb, ec, nm_need):
        u = n["ppu"]
        n["ppu"] += 1
        yg = n["yp"]
        em.wge(PE, sWO, 16)
        em.wge(PE, sNM, nm_need)
        em.wge(PE, sPP, u - 1)
        for fb in range(2):
            mm = PE.matmul(
                pp[u % 2][:, :],
                at_s[:, fb * T + tb * 128: fb * T + (tb + 1) * 128],
                wo_s[:, fb * D + ec * 512: fb * D + (ec + 1) * 512],
                start=(fb == 0), stop=(fb == 1))
        mm.then_inc(sYP, 1)
        n["yp"] += 1
        # DVE y copy
        em.wge(DVE, sYP, n["yp"])
        em.wge(DVE, sST, 16 * (yg - 1))
        DVE.tensor_copy(
            y_s[:, (yg % 2) * 512:(yg % 2) * 512 + 512], pp[u % 2][:, :]
        ).then_inc(sPP, 1)
        n["ppu_copies"] += 1
        # POOL store
        em.wge(POOL, sPP, n["ppu_copies"])
        POOL.dma_start(
            y_d[tb * 128:(tb + 1) * 128, ec * 512:(ec + 1) * 512],
            y_s[:, (yg % 2) * 512:(yg % 2) * 512 + 512],
        ).then_inc(sST, 16)
        n["st"] += 1

    def pop_fill(k=1):
        for _ in range(k):
            if fill_q:
                fill_q.pop(0)[1]()

    # ------------------------------------------------------- attention loops
    def a_phase(pair, J):
        qb, kb = pair, 2 + pair
        nG = 4 * J + 4          # one group per tk-block P
        g0 = gidx[0]
        nm_loop_start = n["nm"]
        dvals = [max(0, 128 * i - 512 * J) for i in range(nG)]
        for P in range(nG):
            ex_after_g[g0 + P] = n["ex"] + P + 1

        def pe_scores_wrap(P):
            d = dvals[P]
            diag = P >= 4 * J
            if diag:
                em.wge(PE, sMQ, 16)
            for hh in range(2):
                ob = d if hh == 0 else 512
                mm = PE.matmul(
                    ps2[P % 2][:, ob: ob + 512 - d],
                    qk_s[hh * 64:(hh + 1) * 64,
                         kb * T + P * 128: kb * T + (P + 1) * 128],
                    qk_s[hh * 64:(hh + 1) * 64,
                         qb * T + J * 512 + d: qb * T + (J + 1) * 512],
                    start=True, stop=not diag, tile_position=(hh * 64, 0),
                )
            if diag:
                for hh in range(2):
                    ob = d if hh == 0 else 512
                    mm = PE.matmul(
                        ps2[P % 2][:, ob: ob + 128],
                        msk_s[:, 0:128], msk_s[:, 128:256],
                        start=False, stop=(hh == 1))
            mm.then_inc(sSC, 1)
            n["sc"] += 1

        def pe_pv(P):
            d = dvals[P]
            slot = ((g0 + P) % 6) * 1024
            em.wge(PE, sEX, ex_after_g[g0 + P])
            em.wge(PE, sPP, v_done[P])
            for hh in range(2):
                h = 2 * pair + hh
                eb = (slot + d) if hh == 0 else (slot + 512)
                if P == 0:
                    em.wge(PE, sNM, nm_prev[hh])
                PE.matmul(
                    pa[hh][:, d:512],
                    vo_s[:, P * 512 + h * 128: P * 512 + (h + 1) * 128],
                    ex_s[:, eb: eb + 512 - d],
                    start=(P == 0), stop=(P == nG - 1),
                ).then_inc(sPV, 1)
                n["pv"] += 1
            pv_after_g[g0 + P] = n["pv"]

        for P in range(nG):
            # ensure the v block for the PV in this burst is scheduled
            if P >= 3:
                while (P - 3) not in v_done:
                    assert fill_q
                    pop_fill()
            # burst waits (one set), then back-to-back MMs
            em.wge(PE, sPP, qk_done[J])
            em.wge(PE, sEX, ex_after_g.get(g0 + P - 2, 0))
            pe_scores_wrap(P)
            if P >= 3:
                pe_pv(P - 3)
            pop_fill()
            # ACT exp for group P: one contiguous span [d, 1024-d)
            d = dvals[P]
            slot = ((g0 + P) % 6) * 1024
            em.wge(ACT, sSC, n["sc"])
            em.wge(ACT, sPV, pv_after_g.get(g0 + P - 6, 0))
            ACT.activation(
                ex_s[:, slot + d: slot + 1024 - d],
                ps2[P % 2][:, d:1024 - d], AF.Exp, scale=float(SCALE),
            ).then_inc(sEX, 1)
            n["ex"] += 1
        for P in range(max(0, nG - 3), nG):
            while P not in v_done:
                assert fill_q
                pop_fill()
            pe_pv(P)
        # dummy recip with no waits: forces the recip table switch to run
        # on ACT concurrently with the PV tail instead of after it
        act_recip(scr_s[0:1, 0:1], scr_s[0:1, 1:2])
        # cover the recip+stt chain at the pair boundary with extra PE work
        pop_fill(3)
        gidx[0] += nG

        # tail: ACT recip + DVE normalize, interleaved per hh so the chain
        # recip(hh0) -> stt(hh0) -> next-phase PV(hh0) is as short as possible
        em.wge(ACT, sNM, nm_loop_start)   # rz_s free of previous loop's stt
        rc_at = {}
        for hh in range(2):
            em.wge(ACT, sPV, n["pv"] - (1 if hh == 0 else 0))
            act_recip(rz_s[hh * 64:(hh + 1) * 64, :],
                      pa[hh][64:128, :]).then_inc(sRC, 1)
            n["rc"] += 1
            rc_at[hh] = n["rc"]
        fb = pair
        for hh in range(2):
            em.wge(DVE, sRC, rc_at[hh])
            DVE.scalar_tensor_tensor(
                at_s[hh * 64:(hh + 1) * 64,
                     fb * T + J * 512: fb * T + (J + 1) * 512],
                pa[hh][0:64, :], 1.0, rz_s[hh * 64:(hh + 1) * 64, :],
                op0=ALU.mult, op1=ALU.mult).then_inc(sNM, 1)
            n["nm"] += 1
            nm_prev[hh] = n["nm"]
        nm_after[(pair, J)] = n["nm"]

    # -------------------------------------------------------------- schedule
    def push_q_chunk(J):
        qk_left[J] = 4
        for eb in range(4):
            for half in range(2):
                fill_q.append((("chunk", J),
                    lambda J=J, eb=eb, h=half: emit_q_half(J, "qk", eb, h)))
        if J == 0:
            # vo ones memset sits in the DVE stream after chunk 0's qk-bias
            # copies (so it doesn't delay them) but before any v copy
            fill_q.append((("chunk", 0),
                           lambda: DVE.memset(vo_s[:, :], 1.0)))
        for tbl in range(4):
            tb = 4 * J + tbl
            for half in range(2):
                fill_q.append((("chunk", J),
                    lambda J=J, tb=tb, h=half: emit_q_half(J, "v", tb, h)))

    def drain_until_qk(J):
        while J not in qk_done:
            pop_fill()

    for J in range(NJ):
        push_q_chunk(J)

    for J in range(NJ):
        drain_until_qk(J)
        for pair in range(2):
            a_phase(pair, J)
        nm_need = nm_after[(1, J)]
        # out-proj groups for J are runnable once the next chunk's proj is
        # done -- insert them after chunk J+1's closures so the inter-phase
        # drain never blocks on the not-yet-ready sNM wait
        out_closures = []
        for tbl in range(4):
            tb = 4 * J + tbl
            for ec in range(2):
                out_closures.append((("out", J),
                    lambda J=J, tb=tb, ec=ec, nm_need=nm_need:
                    emit_o_group(J, tb, ec, nm_need)))
        ins = 0
        for i, (tag, _) in enumerate(fill_q):
            if tag == ("chunk", J + 1):
                ins = i + 1
        fill_q[ins:ins] = out_closures
    while fill_q:
        pop_fill()

    ctx.close()
    return nc


_NC_CACHE = None


def _get_nc():
    global _NC_CACHE
    if _NC_CACHE is None:
        _NC_CACHE = build_nc()
    return _NC_CACHE


def _prep_in_maps(x, W_qkv, b_qkv, W_out, b_out):
    step = np.triu(np.ones((128, 128), np.float32), 1)      # [p<i] -> 1
    negd = np.eye(128, dtype=np.float32) * NEG
    msk = np.concatenate([step, negd], axis=1).astype(bf)   # [128, 256]
    in_maps = []
    for core in range(N_CORES):
        b = core // 4
        heads = [4 * (core % 4) + j for j in range(HPC)]
        fcols = np.concatenate([np.arange(h * HD, (h + 1) * HD) for h in heads])
        # reference packs W_qkv rows per head: head h = rows h*3hd + [q|k|v]
        qrows = np.concatenate(
            [np.arange(h * 3 * HD, h * 3 * HD + HD) for h in heads])
        krows = qrows + HD
        vrows = qrows + 2 * HD
        wqk = np.ascontiguousarray(
            W_qkv[np.concatenate([qrows, krows]), :].T).astype(bf)   # [D, 512]
        wv = np.ascontiguousarray(W_qkv[vrows, :].T).astype(bf)      # [D, 256]
        wo = np.ascontiguousarray(W_out[:, fcols].T).astype(bf)      # [256, D]
        bqk = np.ascontiguousarray(
            b_qkv[np.concatenate([qrows, krows])].reshape(4, 128).T
        ).astype(np.float32)                                          # [128, 4]
        in_maps.append({
            "xt": np.ascontiguousarray(x[b].T).astype(bf),            # [D, T]
            "wqk": wqk, "wv": wv, "wo": wo, "bqk": bqk, "msk": msk,
        })
    return in_maps


def _gather(results, b_qkv, W_out, b_out):
    out = np.zeros((B, T, D), np.float32)
    for core in range(N_CORES):
        out[core // 4] += np.asarray(results[core]["y"], np.float32)
    vidx = np.concatenate(
        [np.arange(h * 3 * HD + 2 * HD, (h + 1) * 3 * HD) for h in range(H)])
    b_v = b_qkv[vidx]
    out += b_out + b_v @ W_out.T
    return out


def _as_f32(*arrs):
    return [np.asarray(a, np.float32) for a in arrs]


def kernel(x, W_qkv, b_qkv, W_out, b_out):
    x, W_qkv, b_qkv, W_out, b_out = _as_f32(x, W_qkv, b_qkv, W_out, b_out)
    in_maps = _prep_in_maps(x, W_qkv, b_qkv, W_out, b_out)
    res = run_bass_kernel_spmd(_get_nc(), in_maps,
                               core_ids=list(range(N_CORES)))
    return _gather(res.results, b_qkv, W_out, b_out)


def run_traced(inputs, trace_cores=None):
    x, W_qkv, b_qkv, W_out, b_out = _as_f32(
        inputs["x"], inputs["W_qkv"], inputs["b_qkv"],
        inputs["W_out"], inputs["b_out"])
    in_maps = _prep_in_maps(x, W_qkv, b_qkv, W_out, b_out)
    res = run_bass_kernel_spmd(_get_nc(), in_maps,
                               core_ids=list(range(N_CORES)),
                               trace=True, trace_cores=trace_cores)
    res.gathered = _gather(res.results, b_qkv, W_out, b_out)
    return res
